# revision 18
# baseline (speedup 1.0000x reference)
"""Trainium2 Bass kernel for nn_Compression_module (dense transformer block).

Full-input contract: kernel(**inputs) takes the unsharded numpy inputs and
returns the full [16, 1024, 512] output. Internally shards data-parallel over
batch across 8 NeuronCores (2 batches/core), runs one SPMD Bass program via
run_bass_kernel_spmd, and concatenates the per-core outputs.
"""
import sys
sys.path.insert(0, '/opt/trn_rl_repo')

from contextlib import ExitStack

import ml_dtypes
import numpy as np

import concourse.bass as bass
import concourse.mybir as mybir
import concourse.tile as tile
from concourse import bacc, bass_utils

# Problem shapes (hardcoded per spec).
B, N, C = 16, 1024, 768
H, KQ, VD = 8, 256, 512
D_OUT = 512
EPS = 1e-5
SCALE = D_OUT ** -0.5
NCORES = 8
BPC = B // NCORES          # batches per core
T = BPC * N                # tokens per core (2048)

F32 = mybir.dt.float32
F32R = mybir.dt.float32r
BF16 = mybir.dt.bfloat16
ADD = mybir.AluOpType.add
MULT = mybir.AluOpType.mult
MIN = mybir.AluOpType.min
MAX = mybir.AluOpType.max
EXP = mybir.ActivationFunctionType.Exp
IDENT = mybir.ActivationFunctionType.Identity

_CACHE = {}


def _build():
    nc = bacc.Bacc("TRN2", target_bir_lowering=False, debug=False,
                   enable_asserts=False)
    xT_d = nc.dram_tensor("xT", [C, T], BF16, kind="ExternalInput")
    wqkT_d = nc.dram_tensor("wqkT", [C, 4 * N], BF16, kind="ExternalInput")
    wvT_d = nc.dram_tensor("wvT", [C, 4 * N], BF16, kind="ExternalInput")
    bqk_d = nc.dram_tensor("bqk", [128, 32], F32, kind="ExternalInput")
    bv_d = nc.dram_tensor("bv", [1, 4 * N], BF16, kind="ExternalInput")
    posT_d = nc.dram_tensor("posT", [H, N, N], BF16, kind="ExternalInput")
    projT_d = nc.dram_tensor("projT", [4 * N, 512], BF16, kind="ExternalInput")
    bproj_d = nc.dram_tensor("bproj", [1, 512], BF16, kind="ExternalInput")
    out_d = nc.dram_tensor("out", [T, 512], F32, kind="ExternalOutput")

    with tile.TileContext(nc) as tc:
        _body(tc, xT_d, wqkT_d, wvT_d, bqk_d, bv_d, posT_d, projT_d, bproj_d,
              out_d)
    nc.compile()
    return nc


def _body(tc, xT_d, wqkT_d, wvT_d, bqk_d, bv_d, posT_d, projT_d, bproj_d,
          out_d):
    nc = tc.nc
    with ExitStack() as top:
        dram = top.enter_context(tc.tile_pool(name="dram", bufs=1, space="DRAM"))
        qkT_s = dram.tile([4 * N, T], BF16, tag="qk")   # feature-major q|k per head
        v_s = dram.tile([T, 4 * N], BF16, tag="v")      # token-major v

        persist = top.enter_context(tc.tile_pool(name="persist", bufs=1))
        bqk_sb = persist.tile([128, 32], F32, tag="bqk")
        nc.sync.dma_start(bqk_sb[:], bqk_d.ap()[:])
        bproj_sb = persist.tile([1, 512], BF16, tag="bproj")
        nc.sync.dma_start(bproj_sb[:], bproj_d.ap()[:])
        ones_f32 = persist.tile([128, 128], F32, tag="onef")
        nc.vector.memset(ones_f32[:], 1.0)
        ones_mat = persist.tile([128, 128], BF16, tag="onem")
        nc.vector.tensor_copy(ones_mat[:], ones_f32[:])
        ones_row = persist.tile([1, 128], BF16, tag="oner")
        nc.vector.tensor_copy(ones_row[:], ones_f32[0:1, :])
        bias_bcast = persist.tile([128, 512], F32, tag="bpb")
        out_acc = [persist.tile([128, 8, 512], F32, tag=f"oacc{b}",
                                name=f"oacc{b}")
                   for b in range(BPC)]

        # proj-bias broadcast to all partitions via ones ⊗ bias matmul
        with tc.tile_pool(name="init_ps", bufs=1, space="PSUM") as ips:
            bb_ps = ips.tile([128, 512], F32, tag="bb")
            nc.tensor.matmul(bb_ps[:], ones_row[:], bproj_sb[:],
                             start=True, stop=True)
            nc.vector.tensor_copy(bias_bcast[:], bb_ps[:])

        # ---------------- Phase A: fused QKV projection ----------------
        with ExitStack() as pa:
            xa = pa.enter_context(tc.tile_pool(name="xa", bufs=1))
            wa = pa.enter_context(tc.tile_pool(name="wa", bufs=2))
            qst = pa.enter_context(tc.tile_pool(name="qst", bufs=3))
            vst = pa.enter_context(tc.tile_pool(name="vst", bufs=3))
            aps = pa.enter_context(
                tc.tile_pool(name="aps", bufs=8, space="PSUM"))

            xT_t = xa.tile([128, 6, T], BF16, tag="x")
            xT_r = xT_d.ap().rearrange("(cc p) t -> p cc t", p=128)
            for th in range(4):
                nc.sync.dma_start(xT_t[:, :, th * 512:(th + 1) * 512],
                                  xT_r[:, :, th * 512:(th + 1) * 512])
            qk_r = qkT_s.rearrange("(fc p) t -> p fc t", p=128)
            wqk_r = wqkT_d.ap().rearrange("(cc p) f -> p cc f", p=128)
            wv_r = wvT_d.ap().rearrange("(cc p) f -> p cc f", p=128)
            v_r = v_s.rearrange("(tc p) f -> p tc f", p=128)

            for fb in range(4):  # q/k feature blocks of 1024
                w_t = wa.tile([128, 6, 1024], BF16, tag="w")
                nc.scalar.dma_start(w_t[:],
                                    wqk_r[:, :, fb * 1024:(fb + 1) * 1024])
                for fs in range(8):
                    fchunk = fb * 8 + fs
                    stage = qst.tile([128, T], BF16, tag="qs")
                    for th in range(T // 512):
                        ps = aps.tile([128, 512], F32, tag="a")
                        for cc in range(6):
                            nc.tensor.matmul(
                                ps[:],
                                w_t[:, cc, fs * 128:(fs + 1) * 128],
                                xT_t[:, cc, th * 512:(th + 1) * 512],
                                start=(cc == 0), stop=(cc == 5))
                        nc.scalar.activation(
                            stage[:, th * 512:(th + 1) * 512], ps[:], IDENT,
                            bias=bqk_sb[:, fchunk:fchunk + 1])
                    nc.sync.dma_start(qk_r[:, fchunk, :], stage[:])

            for fb in range(4):  # v feature blocks of 1024
                w_t = wa.tile([128, 6, 1024], BF16, tag="w")
                nc.scalar.dma_start(w_t[:],
                                    wv_r[:, :, fb * 1024:(fb + 1) * 1024])
                for tcx in range(T // 128):
                    stage = vst.tile([128, 1024], BF16, tag="vs")
                    for fh in range(2):
                        ps = aps.tile([128, 512], F32, tag="a")
                        for cc in range(6):
                            nc.tensor.matmul(
                                ps[:],
                                xT_t[:, cc, tcx * 128:(tcx + 1) * 128],
                                w_t[:, cc, fh * 512:(fh + 1) * 512],
                                start=(cc == 0), stop=(cc == 5))
                        nc.scalar.copy(
                            stage[:, fh * 512:(fh + 1) * 512], ps[:])
                    nc.sync.dma_start(
                        v_r[:, tcx, fb * 1024:(fb + 1) * 1024], stage[:])

        # ---------------- Phase B: attention + fused projection ----------------
        with ExitStack() as pb:
            posp = pb.enter_context(tc.tile_pool(name="posp", bufs=9))
            qp = pb.enter_context(tc.tile_pool(name="qp", bufs=2))
            kp = pb.enter_context(tc.tile_pool(name="kp", bufs=2))
            vp = pb.enter_context(tc.tile_pool(name="vp", bufs=2))
            pjp = pb.enter_context(tc.tile_pool(name="pjp", bufs=2))
            ssb = pb.enter_context(tc.tile_pool(name="ssb", bufs=4))
            esb = pb.enter_context(tc.tile_pool(name="esb", bufs=4))
            osb = pb.enter_context(tc.tile_pool(name="osb", bufs=4))
            clp = pb.enter_context(tc.tile_pool(name="clp", bufs=6))
            smsb = pb.enter_context(tc.tile_pool(name="smsb", bufs=2))
            bvp = pb.enter_context(tc.tile_pool(name="bvp", bufs=2))
            ivp = pb.enter_context(tc.tile_pool(name="ivp", bufs=2))
            sps = pb.enter_context(tc.tile_pool(name="sps", bufs=2, space="PSUM"))
            otps = pb.enter_context(tc.tile_pool(name="otps", bufs=4, space="PSUM"))
            smps = pb.enter_context(tc.tile_pool(name="smps", bufs=1, space="PSUM"))
            pjps = pb.enter_context(tc.tile_pool(name="pjps", bufs=1, space="PSUM"))

            qk_r = qkT_s.rearrange("(fc p) t -> p fc t", p=128)
            v_r = v_s.rearrange("(tc p) f -> p tc f", p=128)
            pj_r = projT_d.ap().rearrange("(c p) f -> p c f", p=128)
            out_r = out_d.ap().rearrange("(tc p) f -> p tc f", p=128)

            for h in range(H):
                pos_t = []
                for kk in range(8):
                    pt = posp.tile([128, N], BF16, tag="pos", name=f"pos{h}_{kk}")
                    nc.sync.dma_start(
                        pt[:],
                        posT_d.ap()[h].rearrange(
                            "(kc p) q -> p kc q", p=128)[:, kk, :])
                    pos_t.append(pt)
                pj_t = pjp.tile([128, 4, 512], BF16, tag="pj")
                nc.sync.dma_start(pj_t[:], pj_r[:, h * 4:(h + 1) * 4, :])
                bv_t = bvp.tile([1, 512], BF16, tag="bvh")
                nc.sync.dma_start(bv_t[:],
                                  bv_d.ap()[:, h * 512:(h + 1) * 512])
                for b in range(BPC):
                    q_t = qp.tile([128, 2, N], BF16, tag="q")
                    nc.sync.dma_start(
                        q_t[:], qk_r[:, h * 4:h * 4 + 2, b * N:(b + 1) * N])
                    k_t = kp.tile([128, 2, N], BF16, tag="k")
                    nc.sync.dma_start(
                        k_t[:], qk_r[:, h * 4 + 2:h * 4 + 4, b * N:(b + 1) * N])
                    v_t = vp.tile([128, 8, 512], BF16, tag="v")
                    nc.sync.dma_start(
                        v_t[:], v_r[:, b * 8:(b + 1) * 8, h * 512:(h + 1) * 512])
                    for qh in range(2):
                        qsl = slice(qh * 512, (qh + 1) * 512)
                        ot_t = [otps.tile([128, 512], F32, tag="ot",
                                           name=f"ot{b}_{h}_{qh}_{i}")
                                for i in range(4)]
                        sm_t = smps.tile([128, 512], F32, tag="sm")
                        for kk in range(8):
                            s_ps = sps.tile([128, 512], F32, tag="s")
                            for d in range(2):
                                nc.tensor.matmul(
                                    s_ps[:],
                                    k_t[:, d, kk * 128:(kk + 1) * 128],
                                    q_t[:, d, qsl],
                                    start=(d == 0), stop=(d == 1))
                            er_t = ssb.tile([128, 512], BF16, tag="er")
                            nc.scalar.activation(er_t[:], s_ps[:], EXP)
                            e_t = esb.tile([128, 512], BF16, tag="e")
                            nc.vector.tensor_tensor(
                                e_t[:], er_t[:], pos_t[kk][:, qsl], MULT)
                            for dv in range(4):
                                nc.tensor.matmul(
                                    ot_t[dv][:],
                                    v_t[:, kk, dv * 128:(dv + 1) * 128],
                                    e_t[:],
                                    start=(kk == 0), stop=False)
                            nc.tensor.matmul(sm_t[:], ones_mat[:], e_t[:],
                                             start=(kk == 0), stop=(kk == 7))
                        sums_sb = smsb.tile([1, 512], BF16, tag="sums")
                        nc.scalar.copy(sums_sb[:], sm_t[0:1, :])
                        inv_sb = ivp.tile([128, 512], F32, tag="inv")
                        nc.vector.reciprocal_approx_fast(inv_sb[:], sm_t[:])
                        cl_t = []
                        for dv in range(4):
                            nc.tensor.matmul(
                                ot_t[dv][:],
                                bv_t[:, dv * 128:(dv + 1) * 128],
                                sums_sb[:],
                                start=False, stop=True)
                            o_sb = osb.tile([128, 512], BF16, tag="o")
                            nc.vector.tensor_tensor(
                                o_sb[:], ot_t[dv][:], inv_sb[:], MULT)
                            cl = clp.tile([128, 512], BF16, tag="cl",
                                          name=f"cl{b}_{h}_{qh}_{dv}")
                            nc.vector.tensor_scalar(
                                cl[:], o_sb[:], 1.0, -1.0, MIN, MAX)
                            cl_t.append(cl)
                        for tcl in range(4):
                            pj_ps = pjps.tile([128, 512], F32, tag="pp")
                            for dv in range(4):
                                nc.tensor.matmul(
                                    pj_ps[:],
                                    cl_t[dv][:, tcl * 128:(tcl + 1) * 128],
                                    pj_t[:, dv, :],
                                    start=(dv == 0), stop=(dv == 3))
                            opj = osb.tile([128, 512], F32, tag="opj",
                                           name=f"opj{b}_{h}_{qh}_{tcl}")
                            nc.scalar.copy(opj[:], pj_ps[:])
                            accs = out_acc[b][:, qh * 4 + tcl, :]
                            prev = bias_bcast[:] if h == 0 else accs
                            nc.gpsimd.tensor_tensor(accs, opj[:], prev, ADD)
                            if h == H - 1:
                                nc.sync.dma_start(
                                    out_r[:, b * 8 + qh * 4 + tcl, :], accs)


def _prep_host(inputs):
    x = np.ascontiguousarray(inputs["x"], dtype=np.float32)
    qkv_w = np.asarray(inputs["qkv_w"], dtype=np.float32)
    g = np.asarray(inputs["qkv_gamma"], np.float32) / np.sqrt(
        np.asarray(inputs["qkv_var"], np.float32) + EPS)
    W = qkv_w * g[:, None]
    bias = (np.asarray(inputs["qkv_beta"], np.float32)
            - np.asarray(inputs["qkv_mean"], np.float32) * g)
    W3 = W.reshape(H, 2 * KQ + VD, C)
    b3 = bias.reshape(H, 2 * KQ + VD)
    wq = W3[:, :KQ] * np.float32(SCALE)
    bq = b3[:, :KQ] * np.float32(SCALE)
    wk, bk = W3[:, KQ:2 * KQ], b3[:, KQ:2 * KQ]
    wv, bv = W3[:, 2 * KQ:], b3[:, 2 * KQ:]
    wqkT = np.ascontiguousarray(
        np.concatenate([wq, wk], axis=1).reshape(4 * N, C).T
    ).astype(ml_dtypes.bfloat16)
    wvT = np.ascontiguousarray(wv.reshape(4 * N, C).T).astype(ml_dtypes.bfloat16)
    bqk2d = np.ascontiguousarray(
        np.concatenate([bq, bk], axis=1).reshape(32, 128).T)
    bvrow = np.ascontiguousarray(bv.reshape(1, 4 * N)).astype(ml_dtypes.bfloat16)
    posT = np.ascontiguousarray(
        np.exp(np.asarray(inputs["pos_bias"], np.float32)).transpose(0, 2, 1)
    ).astype(ml_dtypes.bfloat16)
    gp = np.asarray(inputs["proj_gamma"], np.float32) / np.sqrt(
        np.asarray(inputs["proj_var"], np.float32) + EPS)
    projT = np.ascontiguousarray(
        (np.asarray(inputs["proj_w"], np.float32) * gp[:, None]).T
    ).astype(ml_dtypes.bfloat16)
    bproj = np.ascontiguousarray(
        (np.asarray(inputs["proj_beta"], np.float32)
         - np.asarray(inputs["proj_mean"], np.float32) * gp).reshape(1, 512)
    ).astype(ml_dtypes.bfloat16)

    shared = dict(wqkT=wqkT, wvT=wvT, bqk=bqk2d, bv=bvrow, posT=posT,
                  projT=projT, bproj=bproj)
    in_maps = []
    xs = x.reshape(NCORES, BPC * N, C)
    for i in range(NCORES):
        m = dict(shared)
        m["xT"] = np.ascontiguousarray(xs[i].T).astype(ml_dtypes.bfloat16)
        in_maps.append(m)
    return in_maps


def _run(inputs, trace=False, tmpdir=None):
    if "nc" not in _CACHE:
        _CACHE["nc"] = _build()
    nc = _CACHE["nc"]
    in_maps = _prep_host(inputs)
    res = bass_utils.run_bass_kernel_spmd(
        nc, in_maps, core_ids=list(range(NCORES)), trace=trace, tmpdir=tmpdir)
    out = np.concatenate(
        [r["out"].reshape(BPC, N, D_OUT) for r in res.results], axis=0)
    return out, res


def kernel(**inputs) -> np.ndarray:
    out, _ = _run(inputs)
    return out
